# revision 9
# baseline (speedup 1.0000x reference)
"""MoE layer (8 experts, top-2, capacity 1280) on 8 Trainium2 NeuronCores.

Sharding: expert-parallel. The router (softmax/top-k/position bookkeeping,
~0.3% of FLOPs) runs on host exactly mirroring the reference ops; the
dispatched rows are packed tightly per expert on host (we hold the full
input anyway, so no all-to-all is needed) and core e runs expert e's SwiGLU
FFN over its [padded_rows, d_model] buffer — perfectly load balanced, and
only real routed rows (rounded up to 128) are computed instead of the full
2*capacity zero-padded buffer. Matmuls run in bf16 with fp32 PSUM
accumulation.
"""

import os

import numpy as np
import ml_dtypes

D_MODEL = 1024
D_FF = 4096
E = 8
TOP_K = 2
CAP = 1280  # int(8192 / 8 * 1.25)
N_TOK = 8192
FC = D_FF // 128  # 32 f-chunks
DC = D_MODEL // 128  # 8 d-chunks

BF16 = ml_dtypes.bfloat16

_NC_CACHE = {}  # padded_rows -> compiled Bass program


def _blocks_for(pr):
    # Blocks must be multiples of 128 (stage B) and ideally >= 256 wide:
    # a narrow block restreams the full 16.8MB of w1/w3 for little PE
    # work and becomes weight-DMA-bound (PE starves, HAM re-throttles).
    n512, rem = divmod(pr, 512)
    if rem == 0:
        return [512] * n512
    if rem == 128 and n512 >= 1:
        return [512] * (n512 - 1) + [384, 256]
    return [512] * n512 + [rem]


def _build_program(pr):
    import concourse.bacc as bacc
    import concourse.mybir as mybir
    import concourse.tile as tile

    f32 = mybir.dt.float32
    bf16 = mybir.dt.bfloat16
    blocks = _blocks_for(pr)

    nc = bacc.Bacc("TRN2", target_bir_lowering=False, debug=False, num_devices=E)
    xt_d = nc.dram_tensor("xt", [D_MODEL, pr], bf16, kind="ExternalInput")
    w13_d = nc.dram_tensor("w13", [2, FC, 128, D_MODEL], bf16, kind="ExternalInput")
    w2t_d = nc.dram_tensor("w2t", [D_FF, D_MODEL], bf16, kind="ExternalInput")
    ob_d = nc.dram_tensor("ob", [pr, D_MODEL], f32, kind="ExternalOutput")

    with tile.TileContext(nc) as tc:
        with (
            tc.tile_pool(name="pxt", bufs=1) as pxt,
            tc.tile_pool(name="pw2", bufs=1) as pw2,
            tc.tile_pool(name="ph", bufs=1) as ph,
            tc.tile_pool(name="pw", bufs=3) as pw,
            tc.tile_pool(name="ps", bufs=2) as ps,
            tc.tile_pool(name="po", bufs=3) as po,
            tc.tile_pool(name="pps", bufs=2, space="PSUM") as pps,
            tc.tile_pool(name="ppo", bufs=2, space="PSUM") as ppo,
        ):
            # Resident: dispatched tokens transposed, [p, dc, c].
            # Loaded in per-token-block strips so the first matmul isn't
            # gated on the full transfer.
            xtsb = pxt.tile([128, DC, pr], bf16)
            xt_src = xt_d.ap().rearrange("(a p) c -> p a c", p=128)

            def load_xt_strip(c0, w, split=False):
                if split:
                    # per-dc transfers: lets the first accumulation group's
                    # matmul(dc) start as soon as slice dc lands
                    for dc in range(DC):
                        nc.sync.dma_start(
                            xtsb[:, dc, c0 : c0 + w],
                            xt_src[:, dc, c0 : c0 + w],
                        )
                else:
                    nc.sync.dma_start(
                        xtsb[:, :, c0 : c0 + w], xt_src[:, :, c0 : c0 + w]
                    )

            # Resident: w2^T, [p, fc, d] — trickled in during block 0 stage A
            # (first needed at block 0 stage B).
            w2sb = pw2.tile([128, FC, D_MODEL], bf16)
            w2_src = w2t_d.ap().rearrange("(a p) d -> p a d", p=128)

            load_xt_strip(0, blocks[0], split=True)
            c0 = 0
            for bi, W in enumerate(blocks):
                if bi + 1 < len(blocks):
                    load_xt_strip(c0 + W, blocks[bi + 1])
                h = ph.tile([128, FC, 512], bf16)
                for fc in range(FC):
                    if bi == 0 and 8 <= fc < 16:
                        s = fc - 8
                        nc.sync.dma_start(
                            w2sb[:, s * 4 : (s + 1) * 4, :],
                            w2_src[:, s * 4 : (s + 1) * 4, :],
                        )
                    w1t = pw.tile([128, D_MODEL], bf16)
                    nc.sync.dma_start(w1t[:], w13_d.ap()[0, fc])
                    w3t = pw.tile([128, D_MODEL], bf16)
                    nc.sync.dma_start(w3t[:], w13_d.ap()[1, fc])
                    p1 = pps.tile([128, 512], f32)
                    p3 = pps.tile([128, 512], f32)
                    for dc in range(DC):
                        nc.tensor.matmul(
                            p1[:, :W],
                            w1t[:, dc * 128 : (dc + 1) * 128],
                            xtsb[:, dc, c0 : c0 + W],
                            start=(dc == 0),
                            stop=(dc == DC - 1),
                        )
                    for dc in range(DC):
                        nc.tensor.matmul(
                            p3[:, :W],
                            w3t[:, dc * 128 : (dc + 1) * 128],
                            xtsb[:, dc, c0 : c0 + W],
                            start=(dc == 0),
                            stop=(dc == DC - 1),
                        )
                    s = ps.tile([128, 512], f32)
                    nc.scalar.activation(
                        s[:, :W], p1[:, :W], mybir.ActivationFunctionType.Silu
                    )
                    nc.vector.tensor_mul(h[:, fc, :W], s[:, :W], p3[:, :W])
                # Stage B: ob[t, d] = sum_f h[f, t] * w2t[f, d]
                for ts4 in range(W // 128):
                    for dh in range(2):
                        pob = ppo.tile([128, 512], f32)
                        for fc in range(FC):
                            nc.tensor.matmul(
                                pob[:],
                                h[:, fc, ts4 * 128 : (ts4 + 1) * 128],
                                w2sb[:, fc, dh * 512 : (dh + 1) * 512],
                                start=(fc == 0),
                                stop=(fc == FC - 1),
                            )
                        ot = po.tile([128, 512], f32)
                        nc.scalar.copy(ot[:], pob[:])
                        r0 = c0 + ts4 * 128
                        nc.sync.dma_start(
                            ob_d.ap()[r0 : r0 + 128, dh * 512 : (dh + 1) * 512],
                            ot[:],
                        )
                c0 += W

    nc.compile()
    return nc


def _router_host(xf, gate_w):
    """Router math, mirroring the reference ops on jax-CPU for exactness."""
    try:
        import jax
        import jax.numpy as jnp

        cpu = jax.devices("cpu")[0]
        with jax.default_device(cpu):
            router_logits = jnp.asarray(xf) @ jnp.asarray(gate_w).T
            router_probs = jax.nn.softmax(router_logits, axis=-1)
            top_probs, top_idx = jax.lax.top_k(router_probs, TOP_K)
            top_w = top_probs / (top_probs.sum(-1, keepdims=True) + 1e-10)

            erange = jnp.arange(E, dtype=top_idx.dtype)
            counts = jnp.sum(
                (top_idx[..., None] == erange[None, None, :]).astype(jnp.float32),
                axis=(0, 1),
            )
            f = counts / (N_TOK * TOP_K)
            P = router_probs.mean(axis=0)
            aux_loss = E * jnp.sum(f * P)
        return (
            np.asarray(top_idx),
            np.asarray(top_w),
            np.asarray(aux_loss),
        )
    except Exception:
        # numpy fallback (same math; top-k ties broken by lowest index)
        logits = xf @ np.asarray(gate_w, np.float32).T
        z = logits - logits.max(-1, keepdims=True)
        ez = np.exp(z)
        probs = ez / ez.sum(-1, keepdims=True)
        order = np.argsort(-probs, axis=-1, kind="stable")
        top_idx = order[:, :TOP_K].astype(np.int32)
        top_probs = np.take_along_axis(probs, top_idx, axis=-1)
        top_w = top_probs / (top_probs.sum(-1, keepdims=True) + 1e-10)
        counts = np.bincount(top_idx.ravel(), minlength=E).astype(np.float32)
        f = counts / (N_TOK * TOP_K)
        P = probs.mean(axis=0)
        aux_loss = np.float32(E * np.sum(f * P))
        return top_idx, top_w.astype(np.float32), aux_loss


def _install_trace_shim():
    """Dev-only: register the NTFF profile hook (missing antenv.axon_hooks)
    so run_bass_kernel_spmd(trace=True) can capture HW exec time under axon.
    Returns True if tracing is usable."""
    try:
        import contextlib
        import ctypes
        import sys
        import types

        import concourse.bass_utils as bu

        try:
            from antenv.axon_hooks import get_axon_ntff_profile_hook  # noqa: F401

            return True  # real hooks present
        except ImportError:
            pass

        lib = ctypes.CDLL("/opt/axon/libaxon_pjrt.so")
        if not hasattr(lib, "axon_start_nrt_profile"):
            return False
        lib.axon_start_nrt_profile.argtypes = [
            ctypes.POINTER(ctypes.c_int64),
            ctypes.c_size_t,
        ]
        lib.axon_start_nrt_profile.restype = ctypes.c_int64
        lib.axon_stop_nrt_profile.argtypes = [ctypes.c_char_p]
        lib.axon_stop_nrt_profile.restype = ctypes.c_int64

        @contextlib.contextmanager
        def hook(output_dir, device_ids):
            import jax

            jax.devices()
            if device_ids:
                ids = (ctypes.c_int64 * len(device_ids))(*device_ids)
                rc = lib.axon_start_nrt_profile(ids, len(device_ids))
            else:
                rc = lib.axon_start_nrt_profile(None, 0)
            if rc != 0:
                raise RuntimeError(f"axon_start_nrt_profile rc={rc}")
            try:
                yield
            finally:
                lib.axon_stop_nrt_profile(str(output_dir).encode())

        mod = types.ModuleType("antenv.axon_hooks")
        mod.get_axon_ntff_profile_hook = lambda: hook
        mod.set_axon_ntff_profile_hook = lambda h: None
        sys.modules["antenv.axon_hooks"] = mod
        bu.upload_artifacts = lambda tmpdir: f"file://{tmpdir}"
        return True
    except Exception:
        return False


def kernel(x, gate_w, w1, w2, w3):
    from concourse.bass_utils import run_bass_kernel_spmd

    B, S, D = x.shape
    xf = np.asarray(x, np.float32).reshape(-1, D)

    top_idx, top_w, aux_loss = _router_host(xf, gate_w)

    # Integer bookkeeping (exact): rank of each token in its expert queue.
    idxs, poss, keeps = [], [], []
    kept_cnt = np.zeros((TOP_K, E), np.int64)
    for k in range(TOP_K):
        idx = top_idx[:, k]
        oh = (idx[:, None] == np.arange(E)[None, :]).astype(np.int32)
        pos = oh.cumsum(0)[np.arange(N_TOK), idx] - 1
        keep = pos < CAP
        kept_cnt[k] = np.bincount(idx[keep], minlength=E)
        idxs.append(idx)
        poss.append(pos)
        keeps.append(keep)

    # Tight per-expert packing: rows [k=0 kept | k=1 kept], padded to 128.
    rows_e = kept_cnt.sum(0)
    pr = int(max(-(-int(rows_e.max()) // 128) * 128, 128))

    packed = np.zeros((E, pr, D), np.float32)
    for k in range(TOP_K):
        idx, pos, keep = idxs[k], poss[k], keeps[k]
        row = pos + (kept_cnt[0][idx] if k == 1 else 0)
        packed[idx[keep], row[keep]] = xf[keep]

    # Per-core device inputs (expert-parallel).
    in_maps = []
    for e in range(E):
        xt = np.ascontiguousarray(packed[e].T).astype(BF16)  # [D, pr]
        w1sh = (
            np.asarray(w1[e], np.float32)
            .reshape(FC, 128, DC, 128)
            .transpose(0, 3, 2, 1)
            .reshape(FC, 128, D_MODEL)
        )
        w3sh = (
            np.asarray(w3[e], np.float32)
            .reshape(FC, 128, DC, 128)
            .transpose(0, 3, 2, 1)
            .reshape(FC, 128, D_MODEL)
        )
        w13 = np.ascontiguousarray(np.stack([w1sh, w3sh])).astype(BF16)
        w2t = np.ascontiguousarray(np.asarray(w2[e], np.float32).T).astype(BF16)
        in_maps.append({"xt": xt, "w13": w13, "w2t": w2t})

    if pr not in _NC_CACHE:
        _NC_CACHE[pr] = _build_program(pr)
    nc = _NC_CACHE[pr]

    trace = os.environ.get("BASS_KERNEL_TRACE") == "1"
    kwargs = {}
    if trace and _install_trace_shim():
        kwargs = {"trace": True, "tmpdir": os.environ.get("BASS_KERNEL_TRACE_DIR")}
    res = run_bass_kernel_spmd(nc, in_maps, list(range(E)), **kwargs)
    if trace:
        print(f"HW exec time: {res.exec_time_ns} ns")

    obs = np.stack([res.results[e]["ob"] for e in range(E)])  # [E, pr, D] f32

    out = np.zeros_like(xf)
    for k in range(TOP_K):
        idx, pos, keep = idxs[k], poss[k], keeps[k]
        row = pos + (kept_cnt[0][idx] if k == 1 else 0)
        row = np.where(keep, row, 0)
        gathered = obs[idx, row]  # [N, D]
        coef = (keep.astype(np.float32) * top_w[:, k])[:, None]
        out += gathered * coef

    output = out.reshape(B, S, D)
    return output, np.float32(aux_loss)


# revision 10
# speedup vs baseline: 1.0121x; 1.0121x over previous
"""MoE layer (8 experts, top-2, capacity 1280) on 8 Trainium2 NeuronCores.

Sharding: expert-parallel. The router (softmax/top-k/position bookkeeping,
~0.3% of FLOPs) runs on host exactly mirroring the reference ops; the
dispatched rows are packed tightly per expert on host (we hold the full
input anyway, so no all-to-all is needed) and core e runs expert e's SwiGLU
FFN over its [padded_rows, d_model] buffer — perfectly load balanced, and
only real routed rows (rounded up to 128) are computed instead of the full
2*capacity zero-padded buffer. Matmuls run in bf16 with fp32 PSUM
accumulation.
"""

import os

import numpy as np
import ml_dtypes

D_MODEL = 1024
D_FF = 4096
E = 8
TOP_K = 2
CAP = 1280  # int(8192 / 8 * 1.25)
N_TOK = 8192
FC = D_FF // 128  # 32 f-chunks
DC = D_MODEL // 128  # 8 d-chunks

BF16 = ml_dtypes.bfloat16

_NC_CACHE = {}  # padded_rows -> compiled Bass program


def _blocks_for(pr):
    # Blocks must be multiples of 128 (stage B) and ideally >= 256 wide:
    # a narrow block restreams the full 16.8MB of w1/w3 for little PE
    # work and becomes weight-DMA-bound (PE starves, HAM re-throttles).
    n512, rem = divmod(pr, 512)
    if rem == 0:
        return [512] * n512
    if rem == 128 and n512 >= 1:
        return [512] * (n512 - 1) + [384, 256]
    return [512] * n512 + [rem]


def _build_program(pr):
    import concourse.bacc as bacc
    import concourse.mybir as mybir
    import concourse.tile as tile

    f32 = mybir.dt.float32
    bf16 = mybir.dt.bfloat16
    blocks = _blocks_for(pr)

    nc = bacc.Bacc("TRN2", target_bir_lowering=False, debug=False, num_devices=E)
    xt_d = nc.dram_tensor("xt", [D_MODEL, pr], bf16, kind="ExternalInput")
    w13_d = nc.dram_tensor("w13", [2, FC, 128, D_MODEL], bf16, kind="ExternalInput")
    w2t_d = nc.dram_tensor("w2t", [D_FF, D_MODEL], bf16, kind="ExternalInput")
    ob_d = nc.dram_tensor("ob", [pr, D_MODEL], f32, kind="ExternalOutput")

    with tile.TileContext(nc) as tc:
        with (
            tc.tile_pool(name="pxt", bufs=1) as pxt,
            tc.tile_pool(name="pw2", bufs=1) as pw2,
            tc.tile_pool(name="ph", bufs=1) as ph,
            tc.tile_pool(name="pw", bufs=3) as pw,
            tc.tile_pool(name="ps", bufs=2) as ps,
            tc.tile_pool(name="po", bufs=3) as po,
            tc.tile_pool(name="pps", bufs=2, space="PSUM") as pps,
            tc.tile_pool(name="ppo", bufs=2, space="PSUM") as ppo,
        ):
            # Resident: dispatched tokens transposed, [p, dc, c].
            # Loaded in per-token-block strips so the first matmul isn't
            # gated on the full transfer.
            xtsb = pxt.tile([128, DC, pr], bf16)
            xt_src = xt_d.ap().rearrange("(a p) c -> p a c", p=128)

            def load_xt_strip(c0, w, split=False):
                if split:
                    # per-dc transfers: lets the first accumulation group's
                    # matmul(dc) start as soon as slice dc lands
                    for dc in range(DC):
                        nc.sync.dma_start(
                            xtsb[:, dc, c0 : c0 + w],
                            xt_src[:, dc, c0 : c0 + w],
                        )
                else:
                    nc.sync.dma_start(
                        xtsb[:, :, c0 : c0 + w], xt_src[:, :, c0 : c0 + w]
                    )

            # Resident: w2^T, [p, fc, d] — trickled in during block 0 stage A
            # (first needed at block 0 stage B).
            w2sb = pw2.tile([128, FC, D_MODEL], bf16)
            w2_src = w2t_d.ap().rearrange("(a p) d -> p a d", p=128)

            load_xt_strip(0, blocks[0])
            c0 = 0
            for bi, W in enumerate(blocks):
                if bi + 1 < len(blocks):
                    load_xt_strip(c0 + W, blocks[bi + 1])
                h = ph.tile([128, FC, 512], bf16)
                for fc in range(FC):
                    if bi == 0 and 8 <= fc < 16:
                        s = fc - 8
                        nc.sync.dma_start(
                            w2sb[:, s * 4 : (s + 1) * 4, :],
                            w2_src[:, s * 4 : (s + 1) * 4, :],
                        )
                    w1t = pw.tile([128, D_MODEL], bf16)
                    nc.sync.dma_start(w1t[:], w13_d.ap()[0, fc])
                    w3t = pw.tile([128, D_MODEL], bf16)
                    nc.sync.dma_start(w3t[:], w13_d.ap()[1, fc])
                    p1 = pps.tile([128, 512], f32)
                    p3 = pps.tile([128, 512], f32)
                    for dc in range(DC):
                        nc.tensor.matmul(
                            p1[:, :W],
                            w1t[:, dc * 128 : (dc + 1) * 128],
                            xtsb[:, dc, c0 : c0 + W],
                            start=(dc == 0),
                            stop=(dc == DC - 1),
                        )
                    for dc in range(DC):
                        nc.tensor.matmul(
                            p3[:, :W],
                            w3t[:, dc * 128 : (dc + 1) * 128],
                            xtsb[:, dc, c0 : c0 + W],
                            start=(dc == 0),
                            stop=(dc == DC - 1),
                        )
                    s = ps.tile([128, 512], f32)
                    nc.scalar.activation(
                        s[:, :W], p1[:, :W], mybir.ActivationFunctionType.Silu
                    )
                    nc.vector.tensor_mul(h[:, fc, :W], s[:, :W], p3[:, :W])
                # Stage B: ob[t, d] = sum_f h[f, t] * w2t[f, d]
                for ts4 in range(W // 128):
                    for dh in range(2):
                        pob = ppo.tile([128, 512], f32)
                        for fc in range(FC):
                            nc.tensor.matmul(
                                pob[:],
                                h[:, fc, ts4 * 128 : (ts4 + 1) * 128],
                                w2sb[:, fc, dh * 512 : (dh + 1) * 512],
                                start=(fc == 0),
                                stop=(fc == FC - 1),
                            )
                        ot = po.tile([128, 512], f32)
                        nc.scalar.copy(ot[:], pob[:])
                        r0 = c0 + ts4 * 128
                        nc.sync.dma_start(
                            ob_d.ap()[r0 : r0 + 128, dh * 512 : (dh + 1) * 512],
                            ot[:],
                        )
                c0 += W

    nc.compile()
    return nc


def _router_host(xf, gate_w):
    """Router math, mirroring the reference ops on jax-CPU for exactness."""
    try:
        import jax
        import jax.numpy as jnp

        cpu = jax.devices("cpu")[0]
        with jax.default_device(cpu):
            router_logits = jnp.asarray(xf) @ jnp.asarray(gate_w).T
            router_probs = jax.nn.softmax(router_logits, axis=-1)
            top_probs, top_idx = jax.lax.top_k(router_probs, TOP_K)
            top_w = top_probs / (top_probs.sum(-1, keepdims=True) + 1e-10)

            erange = jnp.arange(E, dtype=top_idx.dtype)
            counts = jnp.sum(
                (top_idx[..., None] == erange[None, None, :]).astype(jnp.float32),
                axis=(0, 1),
            )
            f = counts / (N_TOK * TOP_K)
            P = router_probs.mean(axis=0)
            aux_loss = E * jnp.sum(f * P)
        return (
            np.asarray(top_idx),
            np.asarray(top_w),
            np.asarray(aux_loss),
        )
    except Exception:
        # numpy fallback (same math; top-k ties broken by lowest index)
        logits = xf @ np.asarray(gate_w, np.float32).T
        z = logits - logits.max(-1, keepdims=True)
        ez = np.exp(z)
        probs = ez / ez.sum(-1, keepdims=True)
        order = np.argsort(-probs, axis=-1, kind="stable")
        top_idx = order[:, :TOP_K].astype(np.int32)
        top_probs = np.take_along_axis(probs, top_idx, axis=-1)
        top_w = top_probs / (top_probs.sum(-1, keepdims=True) + 1e-10)
        counts = np.bincount(top_idx.ravel(), minlength=E).astype(np.float32)
        f = counts / (N_TOK * TOP_K)
        P = probs.mean(axis=0)
        aux_loss = np.float32(E * np.sum(f * P))
        return top_idx, top_w.astype(np.float32), aux_loss


def _install_trace_shim():
    """Dev-only: register the NTFF profile hook (missing antenv.axon_hooks)
    so run_bass_kernel_spmd(trace=True) can capture HW exec time under axon.
    Returns True if tracing is usable."""
    try:
        import contextlib
        import ctypes
        import sys
        import types

        import concourse.bass_utils as bu

        try:
            from antenv.axon_hooks import get_axon_ntff_profile_hook  # noqa: F401

            return True  # real hooks present
        except ImportError:
            pass

        lib = ctypes.CDLL("/opt/axon/libaxon_pjrt.so")
        if not hasattr(lib, "axon_start_nrt_profile"):
            return False
        lib.axon_start_nrt_profile.argtypes = [
            ctypes.POINTER(ctypes.c_int64),
            ctypes.c_size_t,
        ]
        lib.axon_start_nrt_profile.restype = ctypes.c_int64
        lib.axon_stop_nrt_profile.argtypes = [ctypes.c_char_p]
        lib.axon_stop_nrt_profile.restype = ctypes.c_int64

        @contextlib.contextmanager
        def hook(output_dir, device_ids):
            import jax

            jax.devices()
            if device_ids:
                ids = (ctypes.c_int64 * len(device_ids))(*device_ids)
                rc = lib.axon_start_nrt_profile(ids, len(device_ids))
            else:
                rc = lib.axon_start_nrt_profile(None, 0)
            if rc != 0:
                raise RuntimeError(f"axon_start_nrt_profile rc={rc}")
            try:
                yield
            finally:
                lib.axon_stop_nrt_profile(str(output_dir).encode())

        mod = types.ModuleType("antenv.axon_hooks")
        mod.get_axon_ntff_profile_hook = lambda: hook
        mod.set_axon_ntff_profile_hook = lambda h: None
        sys.modules["antenv.axon_hooks"] = mod
        bu.upload_artifacts = lambda tmpdir: f"file://{tmpdir}"
        return True
    except Exception:
        return False


def kernel(x, gate_w, w1, w2, w3):
    from concourse.bass_utils import run_bass_kernel_spmd

    B, S, D = x.shape
    xf = np.asarray(x, np.float32).reshape(-1, D)

    top_idx, top_w, aux_loss = _router_host(xf, gate_w)

    # Integer bookkeeping (exact): rank of each token in its expert queue.
    idxs, poss, keeps = [], [], []
    kept_cnt = np.zeros((TOP_K, E), np.int64)
    for k in range(TOP_K):
        idx = top_idx[:, k]
        oh = (idx[:, None] == np.arange(E)[None, :]).astype(np.int32)
        pos = oh.cumsum(0)[np.arange(N_TOK), idx] - 1
        keep = pos < CAP
        kept_cnt[k] = np.bincount(idx[keep], minlength=E)
        idxs.append(idx)
        poss.append(pos)
        keeps.append(keep)

    # Tight per-expert packing: rows [k=0 kept | k=1 kept], padded to 128.
    rows_e = kept_cnt.sum(0)
    pr = int(max(-(-int(rows_e.max()) // 128) * 128, 128))

    packed = np.zeros((E, pr, D), np.float32)
    for k in range(TOP_K):
        idx, pos, keep = idxs[k], poss[k], keeps[k]
        row = pos + (kept_cnt[0][idx] if k == 1 else 0)
        packed[idx[keep], row[keep]] = xf[keep]

    # Per-core device inputs (expert-parallel).
    in_maps = []
    for e in range(E):
        xt = np.ascontiguousarray(packed[e].T).astype(BF16)  # [D, pr]
        w1sh = (
            np.asarray(w1[e], np.float32)
            .reshape(FC, 128, DC, 128)
            .transpose(0, 3, 2, 1)
            .reshape(FC, 128, D_MODEL)
        )
        w3sh = (
            np.asarray(w3[e], np.float32)
            .reshape(FC, 128, DC, 128)
            .transpose(0, 3, 2, 1)
            .reshape(FC, 128, D_MODEL)
        )
        w13 = np.ascontiguousarray(np.stack([w1sh, w3sh])).astype(BF16)
        w2t = np.ascontiguousarray(np.asarray(w2[e], np.float32).T).astype(BF16)
        in_maps.append({"xt": xt, "w13": w13, "w2t": w2t})

    if pr not in _NC_CACHE:
        _NC_CACHE[pr] = _build_program(pr)
    nc = _NC_CACHE[pr]

    trace = os.environ.get("BASS_KERNEL_TRACE") == "1"
    kwargs = {}
    if trace and _install_trace_shim():
        kwargs = {"trace": True, "tmpdir": os.environ.get("BASS_KERNEL_TRACE_DIR")}
    res = run_bass_kernel_spmd(nc, in_maps, list(range(E)), **kwargs)
    if trace:
        print(f"HW exec time: {res.exec_time_ns} ns")

    obs = np.stack([res.results[e]["ob"] for e in range(E)])  # [E, pr, D] f32

    out = np.zeros_like(xf)
    for k in range(TOP_K):
        idx, pos, keep = idxs[k], poss[k], keeps[k]
        row = pos + (kept_cnt[0][idx] if k == 1 else 0)
        row = np.where(keep, row, 0)
        gathered = obs[idx, row]  # [N, D]
        coef = (keep.astype(np.float32) * top_w[:, k])[:, None]
        out += gathered * coef

    output = out.reshape(B, S, D)
    return output, np.float32(aux_loss)


# revision 12
# speedup vs baseline: 1.0210x; 1.0088x over previous
"""MoE layer (8 experts, top-2, capacity 1280) on 8 Trainium2 NeuronCores.

Sharding: expert-parallel. The router (softmax/top-k/position bookkeeping,
~0.3% of FLOPs) runs on host exactly mirroring the reference ops; the
dispatched rows are packed tightly per expert on host (we hold the full
input anyway, so no all-to-all is needed) and core e runs expert e's SwiGLU
FFN over its [padded_rows, d_model] buffer — perfectly load balanced, and
only real routed rows (rounded up to 128) are computed instead of the full
2*capacity zero-padded buffer. Matmuls run in bf16 with fp32 PSUM
accumulation.
"""

import os

import numpy as np
import ml_dtypes

D_MODEL = 1024
D_FF = 4096
E = 8
TOP_K = 2
CAP = 1280  # int(8192 / 8 * 1.25)
N_TOK = 8192
FC = D_FF // 128  # 32 f-chunks
DC = D_MODEL // 128  # 8 d-chunks

BF16 = ml_dtypes.bfloat16

_NC_CACHE = {}  # padded_rows -> compiled Bass program


def _blocks_for(pr):
    # Blocks must be multiples of 128 (stage B) and ideally >= 256 wide:
    # a narrow block restreams the full 16.8MB of w1/w3 for little PE
    # work and becomes weight-DMA-bound (PE starves, HAM re-throttles).
    n512, rem = divmod(pr, 512)
    if rem == 0:
        return [512] * n512
    if rem == 128 and n512 >= 1:
        return [512] * (n512 - 1) + [384, 256]
    return [512] * n512 + [rem]


def _build_program(pr):
    import concourse.bacc as bacc
    import concourse.mybir as mybir
    import concourse.tile as tile

    f32 = mybir.dt.float32
    bf16 = mybir.dt.bfloat16
    blocks = _blocks_for(pr)

    nc = bacc.Bacc("TRN2", target_bir_lowering=False, debug=False, num_devices=E)
    xt_d = nc.dram_tensor("xt", [D_MODEL, pr], bf16, kind="ExternalInput")
    w13_d = nc.dram_tensor("w13", [2, FC, 128, D_MODEL], bf16, kind="ExternalInput")
    w2t_d = nc.dram_tensor("w2t", [D_FF, D_MODEL], bf16, kind="ExternalInput")
    ob_d = nc.dram_tensor("ob", [pr, D_MODEL], f32, kind="ExternalOutput")

    with tile.TileContext(nc) as tc:
        with (
            tc.tile_pool(name="pxt", bufs=1) as pxt,
            tc.tile_pool(name="pw2", bufs=1) as pw2,
            tc.tile_pool(name="ph", bufs=1) as ph,
            tc.tile_pool(name="pw", bufs=3) as pw,
            tc.tile_pool(name="ps", bufs=2) as ps,
            tc.tile_pool(name="po", bufs=3) as po,
            tc.tile_pool(name="pps", bufs=2, space="PSUM") as pps,
            tc.tile_pool(name="ppo", bufs=2, space="PSUM") as ppo,
        ):
            # Resident: dispatched tokens transposed, [p, dc, c].
            # Loaded in per-token-block strips so the first matmul isn't
            # gated on the full transfer.
            xtsb = pxt.tile([128, DC, pr], bf16)
            xt_src = xt_d.ap().rearrange("(a p) c -> p a c", p=128)

            def load_xt_strip(c0, w, split=False):
                if split:
                    # per-dc transfers: lets the first accumulation group's
                    # matmul(dc) start as soon as slice dc lands
                    for dc in range(DC):
                        nc.sync.dma_start(
                            xtsb[:, dc, c0 : c0 + w],
                            xt_src[:, dc, c0 : c0 + w],
                        )
                else:
                    nc.sync.dma_start(
                        xtsb[:, :, c0 : c0 + w], xt_src[:, :, c0 : c0 + w]
                    )

            # Resident: w2^T, [p, fc, d] — trickled in during block 0 stage A
            # (first needed at block 0 stage B).
            w2sb = pw2.tile([128, FC, D_MODEL], bf16)
            w2_src = w2t_d.ap().rearrange("(a p) d -> p a d", p=128)

            load_xt_strip(0, blocks[0])
            c0 = 0
            for bi, W in enumerate(blocks):
                if bi + 1 < len(blocks):
                    load_xt_strip(c0 + W, blocks[bi + 1])
                h = ph.tile([128, FC, 512], bf16)
                for fc in range(FC):
                    if bi == 0 and 8 <= fc < 16:
                        s = fc - 8
                        nc.sync.dma_start(
                            w2sb[:, s * 4 : (s + 1) * 4, :],
                            w2_src[:, s * 4 : (s + 1) * 4, :],
                        )
                    w1t = pw.tile([128, D_MODEL], bf16)
                    nc.sync.dma_start(w1t[:], w13_d.ap()[0, fc])
                    w3t = pw.tile([128, D_MODEL], bf16)
                    nc.sync.dma_start(w3t[:], w13_d.ap()[1, fc])
                    p1 = pps.tile([128, 512], f32)
                    p3 = pps.tile([128, 512], f32)
                    for dc in range(DC):
                        nc.tensor.matmul(
                            p1[:, :W],
                            w1t[:, dc * 128 : (dc + 1) * 128],
                            xtsb[:, dc, c0 : c0 + W],
                            start=(dc == 0),
                            stop=(dc == DC - 1),
                        )
                    for dc in range(DC):
                        nc.tensor.matmul(
                            p3[:, :W],
                            w3t[:, dc * 128 : (dc + 1) * 128],
                            xtsb[:, dc, c0 : c0 + W],
                            start=(dc == 0),
                            stop=(dc == DC - 1),
                        )
                    s = ps.tile([128, 512], f32)
                    nc.scalar.activation(
                        s[:, :W], p1[:, :W], mybir.ActivationFunctionType.Silu
                    )
                    nc.vector.tensor_mul(h[:, fc, :W], s[:, :W], p3[:, :W])
                # Stage B: ob[t, d] = sum_f h[f, t] * w2t[f, d]
                for ts4 in range(W // 128):
                    for dh in range(2):
                        pob = ppo.tile([128, 512], f32)
                        for fc in range(FC):
                            nc.tensor.matmul(
                                pob[:],
                                h[:, fc, ts4 * 128 : (ts4 + 1) * 128],
                                w2sb[:, fc, dh * 512 : (dh + 1) * 512],
                                start=(fc == 0),
                                stop=(fc == FC - 1),
                            )
                        ot = po.tile([128, 512], f32)
                        nc.scalar.copy(ot[:], pob[:])
                        r0 = c0 + ts4 * 128
                        nc.sync.dma_start(
                            ob_d.ap()[r0 : r0 + 128, dh * 512 : (dh + 1) * 512],
                            ot[:],
                        )
                c0 += W

    nc.compile()
    return nc


def _router_host(xf, gate_w):
    """Router math, mirroring the reference ops on jax-CPU for exactness."""
    try:
        import jax
        import jax.numpy as jnp

        cpu = jax.devices("cpu")[0]
        with jax.default_device(cpu):
            router_logits = jnp.asarray(xf) @ jnp.asarray(gate_w).T
            router_probs = jax.nn.softmax(router_logits, axis=-1)
            top_probs, top_idx = jax.lax.top_k(router_probs, TOP_K)
            top_w = top_probs / (top_probs.sum(-1, keepdims=True) + 1e-10)

            erange = jnp.arange(E, dtype=top_idx.dtype)
            counts = jnp.sum(
                (top_idx[..., None] == erange[None, None, :]).astype(jnp.float32),
                axis=(0, 1),
            )
            f = counts / (N_TOK * TOP_K)
            P = router_probs.mean(axis=0)
            aux_loss = E * jnp.sum(f * P)
        return (
            np.asarray(top_idx),
            np.asarray(top_w),
            np.asarray(aux_loss),
        )
    except Exception:
        # numpy fallback (same math; top-k ties broken by lowest index)
        logits = xf @ np.asarray(gate_w, np.float32).T
        z = logits - logits.max(-1, keepdims=True)
        ez = np.exp(z)
        probs = ez / ez.sum(-1, keepdims=True)
        order = np.argsort(-probs, axis=-1, kind="stable")
        top_idx = order[:, :TOP_K].astype(np.int32)
        top_probs = np.take_along_axis(probs, top_idx, axis=-1)
        top_w = top_probs / (top_probs.sum(-1, keepdims=True) + 1e-10)
        counts = np.bincount(top_idx.ravel(), minlength=E).astype(np.float32)
        f = counts / (N_TOK * TOP_K)
        P = probs.mean(axis=0)
        aux_loss = np.float32(E * np.sum(f * P))
        return top_idx, top_w.astype(np.float32), aux_loss


def _install_trace_shim():
    """Dev-only: register the NTFF profile hook (missing antenv.axon_hooks)
    so run_bass_kernel_spmd(trace=True) can capture HW exec time under axon.
    Returns True if tracing is usable."""
    try:
        import contextlib
        import ctypes
        import sys
        import types

        import concourse.bass_utils as bu

        try:
            from antenv.axon_hooks import get_axon_ntff_profile_hook  # noqa: F401

            return True  # real hooks present
        except ImportError:
            pass

        lib = ctypes.CDLL("/opt/axon/libaxon_pjrt.so")
        if not hasattr(lib, "axon_start_nrt_profile"):
            return False
        lib.axon_start_nrt_profile.argtypes = [
            ctypes.POINTER(ctypes.c_int64),
            ctypes.c_size_t,
        ]
        lib.axon_start_nrt_profile.restype = ctypes.c_int64
        lib.axon_stop_nrt_profile.argtypes = [ctypes.c_char_p]
        lib.axon_stop_nrt_profile.restype = ctypes.c_int64

        @contextlib.contextmanager
        def hook(output_dir, device_ids):
            import jax

            jax.devices()
            if device_ids:
                ids = (ctypes.c_int64 * len(device_ids))(*device_ids)
                rc = lib.axon_start_nrt_profile(ids, len(device_ids))
            else:
                rc = lib.axon_start_nrt_profile(None, 0)
            if rc != 0:
                raise RuntimeError(f"axon_start_nrt_profile rc={rc}")
            try:
                yield
            finally:
                lib.axon_stop_nrt_profile(str(output_dir).encode())

        mod = types.ModuleType("antenv.axon_hooks")
        mod.get_axon_ntff_profile_hook = lambda: hook
        mod.set_axon_ntff_profile_hook = lambda h: None
        sys.modules["antenv.axon_hooks"] = mod
        bu.upload_artifacts = lambda tmpdir: f"file://{tmpdir}"
        return True
    except Exception:
        return False


def kernel(x, gate_w, w1, w2, w3):
    from concourse.bass_utils import run_bass_kernel_spmd

    B, S, D = x.shape
    xf = np.asarray(x, np.float32).reshape(-1, D)

    top_idx, top_w, aux_loss = _router_host(xf, gate_w)

    # Integer bookkeeping (exact): rank of each token in its expert queue.
    idxs, poss, keeps = [], [], []
    kept_cnt = np.zeros((TOP_K, E), np.int64)
    for k in range(TOP_K):
        idx = top_idx[:, k]
        oh = (idx[:, None] == np.arange(E)[None, :]).astype(np.int32)
        pos = oh.cumsum(0)[np.arange(N_TOK), idx] - 1
        keep = pos < CAP
        kept_cnt[k] = np.bincount(idx[keep], minlength=E)
        idxs.append(idx)
        poss.append(pos)
        keeps.append(keep)

    # Tight per-expert packing: rows [k=0 kept | k=1 kept], padded to 128.
    rows_e = kept_cnt.sum(0)
    pr = int(max(-(-int(rows_e.max()) // 128) * 128, 128))

    packed = np.zeros((E, pr, D), np.float32)
    for k in range(TOP_K):
        idx, pos, keep = idxs[k], poss[k], keeps[k]
        row = pos + (kept_cnt[0][idx] if k == 1 else 0)
        packed[idx[keep], row[keep]] = xf[keep]

    # Per-core device inputs (expert-parallel).
    in_maps = []
    for e in range(E):
        xt = np.ascontiguousarray(packed[e].T).astype(BF16)  # [D, pr]
        w1sh = (
            np.asarray(w1[e], np.float32)
            .reshape(FC, 128, DC, 128)
            .transpose(0, 3, 2, 1)
            .reshape(FC, 128, D_MODEL)
        )
        w3sh = (
            np.asarray(w3[e], np.float32)
            .reshape(FC, 128, DC, 128)
            .transpose(0, 3, 2, 1)
            .reshape(FC, 128, D_MODEL)
        )
        w13 = np.ascontiguousarray(np.stack([w1sh, w3sh])).astype(BF16)
        w2t = np.ascontiguousarray(np.asarray(w2[e], np.float32).T).astype(BF16)
        in_maps.append({"xt": xt, "w13": w13, "w2t": w2t})

    if pr not in _NC_CACHE:
        _NC_CACHE[pr] = _build_program(pr)
    nc = _NC_CACHE[pr]

    trace = os.environ.get("BASS_KERNEL_TRACE") == "1"
    kwargs = {}
    if trace and _install_trace_shim():
        kwargs = {"trace": True, "tmpdir": os.environ.get("BASS_KERNEL_TRACE_DIR")}
    res = run_bass_kernel_spmd(nc, in_maps, list(range(E)), **kwargs)
    if trace:
        print(f"HW exec time: {res.exec_time_ns} ns")

    obs = np.stack([res.results[e]["ob"] for e in range(E)])  # [E, pr, D] f32

    out = np.zeros_like(xf)
    for k in range(TOP_K):
        idx, pos, keep = idxs[k], poss[k], keeps[k]
        row = pos + (kept_cnt[0][idx] if k == 1 else 0)
        row = np.where(keep, row, 0)
        gathered = obs[idx, row]  # [N, D]
        coef = (keep.astype(np.float32) * top_w[:, k])[:, None]
        out += gathered * coef

    output = out.reshape(B, S, D)
    return output, np.float32(aux_loss)


# revision 13
# speedup vs baseline: 1.0211x; 1.0001x over previous
"""MoE layer (8 experts, top-2, capacity 1280) on 8 Trainium2 NeuronCores.

Sharding: expert-parallel. The router (softmax/top-k/position bookkeeping,
~0.3% of FLOPs) runs on host exactly mirroring the reference ops; the
dispatched rows are packed tightly per expert on host (we hold the full
input anyway, so no all-to-all is needed) and core e runs expert e's SwiGLU
FFN over its [padded_rows, d_model] buffer — perfectly load balanced, and
only real routed rows (rounded up to 128) are computed instead of the full
2*capacity zero-padded buffer. Matmuls run in bf16 with fp32 PSUM
accumulation.
"""

import os

import numpy as np
import ml_dtypes

D_MODEL = 1024
D_FF = 4096
E = 8
TOP_K = 2
CAP = 1280  # int(8192 / 8 * 1.25)
N_TOK = 8192
FC = D_FF // 128  # 32 f-chunks
DC = D_MODEL // 128  # 8 d-chunks

BF16 = ml_dtypes.bfloat16

_NC_CACHE = {}  # padded_rows -> compiled Bass program


def _blocks_for(pr):
    # Blocks must be multiples of 128 (stage B) and ideally >= 256 wide:
    # a narrow block restreams the full 16.8MB of w1/w3 for little PE
    # work and becomes weight-DMA-bound (PE starves, HAM re-throttles).
    n512, rem = divmod(pr, 512)
    if rem == 0:
        return [512] * n512
    if rem == 128 and n512 >= 1:
        return [512] * (n512 - 1) + [384, 256]
    return [512] * n512 + [rem]


def _build_program(pr):
    import concourse.bacc as bacc
    import concourse.mybir as mybir
    import concourse.tile as tile

    f32 = mybir.dt.float32
    bf16 = mybir.dt.bfloat16
    blocks = _blocks_for(pr)

    nc = bacc.Bacc("TRN2", target_bir_lowering=False, debug=False, num_devices=E)
    xt_d = nc.dram_tensor("xt", [D_MODEL, pr], bf16, kind="ExternalInput")
    w13_d = nc.dram_tensor("w13", [2, FC, 128, D_MODEL], bf16, kind="ExternalInput")
    w2t_d = nc.dram_tensor("w2t", [D_FF, D_MODEL], bf16, kind="ExternalInput")
    ob_d = nc.dram_tensor("ob", [pr, D_MODEL], f32, kind="ExternalOutput")

    with tile.TileContext(nc) as tc:
        with (
            tc.tile_pool(name="pxt", bufs=1) as pxt,
            tc.tile_pool(name="pw2", bufs=1) as pw2,
            tc.tile_pool(name="ph", bufs=1) as ph,
            tc.tile_pool(name="pw", bufs=3) as pw,
            tc.tile_pool(name="ps", bufs=2) as ps,
            tc.tile_pool(name="po", bufs=3) as po,
            tc.tile_pool(name="pps", bufs=2, space="PSUM") as pps,
            tc.tile_pool(name="ppo", bufs=2, space="PSUM") as ppo,
        ):
            # Resident: dispatched tokens transposed, [p, dc, c].
            # Loaded in per-token-block strips so the first matmul isn't
            # gated on the full transfer.
            xtsb = pxt.tile([128, DC, pr], bf16)
            xt_src = xt_d.ap().rearrange("(a p) c -> p a c", p=128)

            def load_xt_strip(c0, w, split=False):
                if split:
                    # per-dc transfers: lets the first accumulation group's
                    # matmul(dc) start as soon as slice dc lands
                    for dc in range(DC):
                        nc.sync.dma_start(
                            xtsb[:, dc, c0 : c0 + w],
                            xt_src[:, dc, c0 : c0 + w],
                        )
                else:
                    nc.sync.dma_start(
                        xtsb[:, :, c0 : c0 + w], xt_src[:, :, c0 : c0 + w]
                    )

            # Resident: w2^T, [p, fc, d] — trickled in during block 0 stage A
            # (first needed at block 0 stage B).
            w2sb = pw2.tile([128, FC, D_MODEL], bf16)
            w2_src = w2t_d.ap().rearrange("(a p) d -> p a d", p=128)

            load_xt_strip(0, blocks[0])
            c0 = 0
            for bi, W in enumerate(blocks):
                if bi + 1 < len(blocks):
                    load_xt_strip(c0 + W, blocks[bi + 1])
                h = ph.tile([128, FC, 512], bf16)
                for fc in range(FC):
                    if bi == 0 and 8 <= fc < 16:
                        s = fc - 8
                        nc.sync.dma_start(
                            w2sb[:, s * 4 : (s + 1) * 4, :],
                            w2_src[:, s * 4 : (s + 1) * 4, :],
                        )
                    w1t = pw.tile([128, D_MODEL], bf16)
                    nc.sync.dma_start(w1t[:], w13_d.ap()[0, fc])
                    w3t = pw.tile([128, D_MODEL], bf16)
                    nc.sync.dma_start(w3t[:], w13_d.ap()[1, fc])
                    p1 = pps.tile([128, 512], f32)
                    p3 = pps.tile([128, 512], f32)
                    for dc in range(DC):
                        nc.tensor.matmul(
                            p1[:, :W],
                            w1t[:, dc * 128 : (dc + 1) * 128],
                            xtsb[:, dc, c0 : c0 + W],
                            start=(dc == 0),
                            stop=(dc == DC - 1),
                        )
                    for dc in range(DC):
                        nc.tensor.matmul(
                            p3[:, :W],
                            w3t[:, dc * 128 : (dc + 1) * 128],
                            xtsb[:, dc, c0 : c0 + W],
                            start=(dc == 0),
                            stop=(dc == DC - 1),
                        )
                    s = ps.tile([128, 512], f32)
                    nc.scalar.activation(
                        s[:, :W], p1[:, :W], mybir.ActivationFunctionType.Silu
                    )
                    nc.vector.tensor_mul(h[:, fc, :W], s[:, :W], p3[:, :W])
                # Stage B: ob[t, d] = sum_f h[f, t] * w2t[f, d]
                for ts4 in range(W // 128):
                    for dh in range(2):
                        pob = ppo.tile([128, 512], f32)
                        for fc in range(FC):
                            nc.tensor.matmul(
                                pob[:],
                                h[:, fc, ts4 * 128 : (ts4 + 1) * 128],
                                w2sb[:, fc, dh * 512 : (dh + 1) * 512],
                                start=(fc == 0),
                                stop=(fc == FC - 1),
                            )
                        ot = po.tile([128, 512], f32)
                        nc.scalar.copy(ot[:], pob[:])
                        r0 = c0 + ts4 * 128
                        nc.sync.dma_start(
                            ob_d.ap()[r0 : r0 + 128, dh * 512 : (dh + 1) * 512],
                            ot[:],
                        )
                c0 += W

    nc.compile()
    return nc


def _router_host(xf, gate_w):
    """Router math, mirroring the reference ops on jax-CPU for exactness."""
    try:
        import jax
        import jax.numpy as jnp

        cpu = jax.devices("cpu")[0]
        with jax.default_device(cpu):
            router_logits = jnp.asarray(xf) @ jnp.asarray(gate_w).T
            router_probs = jax.nn.softmax(router_logits, axis=-1)
            top_probs, top_idx = jax.lax.top_k(router_probs, TOP_K)
            top_w = top_probs / (top_probs.sum(-1, keepdims=True) + 1e-10)

            erange = jnp.arange(E, dtype=top_idx.dtype)
            counts = jnp.sum(
                (top_idx[..., None] == erange[None, None, :]).astype(jnp.float32),
                axis=(0, 1),
            )
            f = counts / (N_TOK * TOP_K)
            P = router_probs.mean(axis=0)
            aux_loss = E * jnp.sum(f * P)
        return (
            np.asarray(top_idx),
            np.asarray(top_w),
            np.asarray(aux_loss),
        )
    except Exception:
        # numpy fallback (same math; top-k ties broken by lowest index)
        logits = xf @ np.asarray(gate_w, np.float32).T
        z = logits - logits.max(-1, keepdims=True)
        ez = np.exp(z)
        probs = ez / ez.sum(-1, keepdims=True)
        order = np.argsort(-probs, axis=-1, kind="stable")
        top_idx = order[:, :TOP_K].astype(np.int32)
        top_probs = np.take_along_axis(probs, top_idx, axis=-1)
        top_w = top_probs / (top_probs.sum(-1, keepdims=True) + 1e-10)
        counts = np.bincount(top_idx.ravel(), minlength=E).astype(np.float32)
        f = counts / (N_TOK * TOP_K)
        P = probs.mean(axis=0)
        aux_loss = np.float32(E * np.sum(f * P))
        return top_idx, top_w.astype(np.float32), aux_loss


def _install_trace_shim():
    """Dev-only: register the NTFF profile hook (missing antenv.axon_hooks)
    so run_bass_kernel_spmd(trace=True) can capture HW exec time under axon.
    Returns True if tracing is usable."""
    try:
        import contextlib
        import ctypes
        import sys
        import types

        import concourse.bass_utils as bu

        try:
            from antenv.axon_hooks import get_axon_ntff_profile_hook  # noqa: F401

            return True  # real hooks present
        except ImportError:
            pass

        lib = ctypes.CDLL("/opt/axon/libaxon_pjrt.so")
        if not hasattr(lib, "axon_start_nrt_profile"):
            return False
        lib.axon_start_nrt_profile.argtypes = [
            ctypes.POINTER(ctypes.c_int64),
            ctypes.c_size_t,
        ]
        lib.axon_start_nrt_profile.restype = ctypes.c_int64
        lib.axon_stop_nrt_profile.argtypes = [ctypes.c_char_p]
        lib.axon_stop_nrt_profile.restype = ctypes.c_int64

        @contextlib.contextmanager
        def hook(output_dir, device_ids):
            import jax

            jax.devices()
            if device_ids:
                ids = (ctypes.c_int64 * len(device_ids))(*device_ids)
                rc = lib.axon_start_nrt_profile(ids, len(device_ids))
            else:
                rc = lib.axon_start_nrt_profile(None, 0)
            if rc != 0:
                raise RuntimeError(f"axon_start_nrt_profile rc={rc}")
            try:
                yield
            finally:
                lib.axon_stop_nrt_profile(str(output_dir).encode())

        mod = types.ModuleType("antenv.axon_hooks")
        mod.get_axon_ntff_profile_hook = lambda: hook
        mod.set_axon_ntff_profile_hook = lambda h: None
        sys.modules["antenv.axon_hooks"] = mod
        bu.upload_artifacts = lambda tmpdir: f"file://{tmpdir}"
        return True
    except Exception:
        return False


def kernel(x, gate_w, w1, w2, w3):
    from concourse.bass_utils import run_bass_kernel_spmd

    global N_TOK, CAP
    B, S, D = x.shape
    xf = np.asarray(x, np.float32).reshape(-1, D)
    N_TOK = xf.shape[0]
    CAP = int(N_TOK / E * 1.25)

    top_idx, top_w, aux_loss = _router_host(xf, gate_w)

    # Integer bookkeeping (exact): rank of each token in its expert queue.
    idxs, poss, keeps = [], [], []
    kept_cnt = np.zeros((TOP_K, E), np.int64)
    for k in range(TOP_K):
        idx = top_idx[:, k]
        oh = (idx[:, None] == np.arange(E)[None, :]).astype(np.int32)
        pos = oh.cumsum(0)[np.arange(N_TOK), idx] - 1
        keep = pos < CAP
        kept_cnt[k] = np.bincount(idx[keep], minlength=E)
        idxs.append(idx)
        poss.append(pos)
        keeps.append(keep)

    # Tight per-expert packing: rows [k=0 kept | k=1 kept], padded to 128.
    rows_e = kept_cnt.sum(0)
    pr = int(max(-(-int(rows_e.max()) // 128) * 128, 128))

    packed = np.zeros((E, pr, D), np.float32)
    for k in range(TOP_K):
        idx, pos, keep = idxs[k], poss[k], keeps[k]
        row = pos + (kept_cnt[0][idx] if k == 1 else 0)
        packed[idx[keep], row[keep]] = xf[keep]

    # Per-core device inputs (expert-parallel).
    in_maps = []
    for e in range(E):
        xt = np.ascontiguousarray(packed[e].T).astype(BF16)  # [D, pr]
        w1sh = (
            np.asarray(w1[e], np.float32)
            .reshape(FC, 128, DC, 128)
            .transpose(0, 3, 2, 1)
            .reshape(FC, 128, D_MODEL)
        )
        w3sh = (
            np.asarray(w3[e], np.float32)
            .reshape(FC, 128, DC, 128)
            .transpose(0, 3, 2, 1)
            .reshape(FC, 128, D_MODEL)
        )
        w13 = np.ascontiguousarray(np.stack([w1sh, w3sh])).astype(BF16)
        w2t = np.ascontiguousarray(np.asarray(w2[e], np.float32).T).astype(BF16)
        in_maps.append({"xt": xt, "w13": w13, "w2t": w2t})

    if pr not in _NC_CACHE:
        _NC_CACHE[pr] = _build_program(pr)
    nc = _NC_CACHE[pr]

    trace = os.environ.get("BASS_KERNEL_TRACE") == "1"
    kwargs = {}
    if trace and _install_trace_shim():
        kwargs = {"trace": True, "tmpdir": os.environ.get("BASS_KERNEL_TRACE_DIR")}
    res = run_bass_kernel_spmd(nc, in_maps, list(range(E)), **kwargs)
    if trace:
        print(f"HW exec time: {res.exec_time_ns} ns")

    obs = np.stack([res.results[e]["ob"] for e in range(E)])  # [E, pr, D] f32

    out = np.zeros_like(xf)
    for k in range(TOP_K):
        idx, pos, keep = idxs[k], poss[k], keeps[k]
        row = pos + (kept_cnt[0][idx] if k == 1 else 0)
        row = np.where(keep, row, 0)
        gathered = obs[idx, row]  # [N, D]
        coef = (keep.astype(np.float32) * top_w[:, k])[:, None]
        out += gathered * coef

    output = out.reshape(B, S, D)
    return output, np.float32(aux_loss)


# revision 17
# speedup vs baseline: 1.0426x; 1.0210x over previous
"""MoE layer (8 experts, top-2, capacity 1280) on 8 Trainium2 NeuronCores.

Sharding: expert-parallel. The router (softmax/top-k/position bookkeeping,
~0.3% of FLOPs) runs on host exactly mirroring the reference ops; the
dispatched rows are packed tightly per expert on host (we hold the full
input anyway, so no all-to-all is needed) and core e runs expert e's SwiGLU
FFN over its [padded_rows, d_model] buffer — perfectly load balanced, and
only real routed rows (rounded up to 128) are computed instead of the full
2*capacity zero-padded buffer. Matmuls run in bf16 with fp32 PSUM
accumulation.
"""

import os

import numpy as np
import ml_dtypes

D_MODEL = 1024
D_FF = 4096
E = 8
TOP_K = 2
CAP = 1280  # int(8192 / 8 * 1.25)
N_TOK = 8192
FC = D_FF // 128  # 32 f-chunks
DC = D_MODEL // 128  # 8 d-chunks

BF16 = ml_dtypes.bfloat16

_NC_CACHE = {}  # padded_rows -> compiled Bass program


def _blocks_for(pr):
    # Blocks should be >= 256 wide: a narrow block restreams the full
    # 16.8MB of w1/w3 for little PE work and becomes weight-DMA-bound
    # (PE starves, HAM re-throttles). Widths need not be multiples of
    # 128 — stage B handles a ragged tail sub-block.
    n512, rem = divmod(pr, 512)
    if rem == 0:
        return [512] * n512
    if rem >= 256 or n512 == 0:
        return [512] * n512 + [rem]
    return [512] * (n512 - 1) + [256, 256 + rem]


def _build_program(pr):
    import concourse.bacc as bacc
    import concourse.mybir as mybir
    import concourse.tile as tile

    f32 = mybir.dt.float32
    bf16 = mybir.dt.bfloat16
    blocks = _blocks_for(pr)

    nc = bacc.Bacc("TRN2", target_bir_lowering=False, debug=False, num_devices=E)
    xt_d = nc.dram_tensor("xt", [D_MODEL, pr], bf16, kind="ExternalInput")
    w13_d = nc.dram_tensor("w13", [2, FC, 128, D_MODEL], bf16, kind="ExternalInput")
    w2t_d = nc.dram_tensor("w2t", [D_FF, D_MODEL], bf16, kind="ExternalInput")
    ob_d = nc.dram_tensor("ob", [pr, D_MODEL], f32, kind="ExternalOutput")

    with tile.TileContext(nc) as tc:
        with (
            tc.tile_pool(name="pxt", bufs=1) as pxt,
            tc.tile_pool(name="pw2", bufs=1) as pw2,
            tc.tile_pool(name="ph", bufs=1) as ph,
            tc.tile_pool(name="pw", bufs=6) as pw,
            tc.tile_pool(name="ps", bufs=2) as ps,
            tc.tile_pool(name="po", bufs=3) as po,
            tc.tile_pool(name="pps", bufs=2, space="PSUM") as pps,
            tc.tile_pool(name="ppo", bufs=2, space="PSUM") as ppo,
        ):
            # Resident: dispatched tokens transposed, [p, dc, c].
            # Loaded in per-token-block strips so the first matmul isn't
            # gated on the full transfer.
            xtsb = pxt.tile([128, DC, pr], bf16)
            xt_src = xt_d.ap().rearrange("(a p) c -> p a c", p=128)

            def load_xt_strip(c0, w, split=False):
                if split:
                    # per-dc transfers: lets the first accumulation group's
                    # matmul(dc) start as soon as slice dc lands
                    for dc in range(DC):
                        nc.sync.dma_start(
                            xtsb[:, dc, c0 : c0 + w],
                            xt_src[:, dc, c0 : c0 + w],
                        )
                else:
                    nc.sync.dma_start(
                        xtsb[:, :, c0 : c0 + w], xt_src[:, :, c0 : c0 + w]
                    )

            # Resident: w2^T, [p, fc, d] — trickled in during block 0 stage A
            # (first needed at block 0 stage B).
            w2sb = pw2.tile([128, FC, D_MODEL], bf16)
            w2_src = w2t_d.ap().rearrange("(a p) d -> p a d", p=128)

            load_xt_strip(0, blocks[0])
            c0 = 0
            for bi, W in enumerate(blocks):
                if bi + 1 < len(blocks):
                    load_xt_strip(c0 + W, blocks[bi + 1])
                h = ph.tile([128, FC, 512], bf16)
                for fc in range(FC):
                    if bi == 0 and 8 <= fc < 16:
                        s = fc - 8
                        nc.sync.dma_start(
                            w2sb[:, s * 4 : (s + 1) * 4, :],
                            w2_src[:, s * 4 : (s + 1) * 4, :],
                        )
                    w1t = pw.tile([128, D_MODEL], bf16)
                    nc.sync.dma_start(w1t[:], w13_d.ap()[0, fc])
                    w3t = pw.tile([128, D_MODEL], bf16)
                    nc.sync.dma_start(w3t[:], w13_d.ap()[1, fc])
                    p1 = pps.tile([128, 512], f32)
                    p3 = pps.tile([128, 512], f32)
                    for dc in range(DC):
                        nc.tensor.matmul(
                            p1[:, :W],
                            w1t[:, dc * 128 : (dc + 1) * 128],
                            xtsb[:, dc, c0 : c0 + W],
                            start=(dc == 0),
                            stop=(dc == DC - 1),
                        )
                    for dc in range(DC):
                        nc.tensor.matmul(
                            p3[:, :W],
                            w3t[:, dc * 128 : (dc + 1) * 128],
                            xtsb[:, dc, c0 : c0 + W],
                            start=(dc == 0),
                            stop=(dc == DC - 1),
                        )
                    s = ps.tile([128, 512], f32)
                    nc.scalar.activation(
                        s[:, :W], p1[:, :W], mybir.ActivationFunctionType.Silu
                    )
                    nc.vector.tensor_mul(h[:, fc, :W], s[:, :W], p3[:, :W])
                # Stage B: ob[t, d] = sum_f h[f, t] * w2t[f, d]
                # (ragged tail sub-block allowed: matmul cost depends on the
                # moving free dim, not the output partition count)
                for ts4 in range(-(-W // 128)):
                    t0 = ts4 * 128
                    tw = min(128, W - t0)
                    for dh in range(2):
                        pob = ppo.tile([128, 512], f32)
                        for fc in range(FC):
                            nc.tensor.matmul(
                                pob[:tw, :],
                                h[:, fc, t0 : t0 + tw],
                                w2sb[:, fc, dh * 512 : (dh + 1) * 512],
                                start=(fc == 0),
                                stop=(fc == FC - 1),
                            )
                        ot = po.tile([128, 512], f32)
                        nc.scalar.copy(ot[:tw, :], pob[:tw, :])
                        r0 = c0 + t0
                        nc.sync.dma_start(
                            ob_d.ap()[r0 : r0 + tw, dh * 512 : (dh + 1) * 512],
                            ot[:tw, :],
                        )
                c0 += W

    nc.compile()
    return nc


def _router_host(xf, gate_w):
    """Router math, mirroring the reference ops on jax-CPU for exactness."""
    try:
        import jax
        import jax.numpy as jnp

        cpu = jax.devices("cpu")[0]
        with jax.default_device(cpu):
            router_logits = jnp.asarray(xf) @ jnp.asarray(gate_w).T
            router_probs = jax.nn.softmax(router_logits, axis=-1)
            top_probs, top_idx = jax.lax.top_k(router_probs, TOP_K)
            top_w = top_probs / (top_probs.sum(-1, keepdims=True) + 1e-10)

            erange = jnp.arange(E, dtype=top_idx.dtype)
            counts = jnp.sum(
                (top_idx[..., None] == erange[None, None, :]).astype(jnp.float32),
                axis=(0, 1),
            )
            f = counts / (N_TOK * TOP_K)
            P = router_probs.mean(axis=0)
            aux_loss = E * jnp.sum(f * P)
        return (
            np.asarray(top_idx),
            np.asarray(top_w),
            np.asarray(aux_loss),
        )
    except Exception:
        # numpy fallback (same math; top-k ties broken by lowest index)
        logits = xf @ np.asarray(gate_w, np.float32).T
        z = logits - logits.max(-1, keepdims=True)
        ez = np.exp(z)
        probs = ez / ez.sum(-1, keepdims=True)
        order = np.argsort(-probs, axis=-1, kind="stable")
        top_idx = order[:, :TOP_K].astype(np.int32)
        top_probs = np.take_along_axis(probs, top_idx, axis=-1)
        top_w = top_probs / (top_probs.sum(-1, keepdims=True) + 1e-10)
        counts = np.bincount(top_idx.ravel(), minlength=E).astype(np.float32)
        f = counts / (N_TOK * TOP_K)
        P = probs.mean(axis=0)
        aux_loss = np.float32(E * np.sum(f * P))
        return top_idx, top_w.astype(np.float32), aux_loss


def _install_trace_shim():
    """Dev-only: register the NTFF profile hook (missing antenv.axon_hooks)
    so run_bass_kernel_spmd(trace=True) can capture HW exec time under axon.
    Returns True if tracing is usable."""
    try:
        import contextlib
        import ctypes
        import sys
        import types

        import concourse.bass_utils as bu

        try:
            from antenv.axon_hooks import get_axon_ntff_profile_hook  # noqa: F401

            return True  # real hooks present
        except ImportError:
            pass

        lib = ctypes.CDLL("/opt/axon/libaxon_pjrt.so")
        if not hasattr(lib, "axon_start_nrt_profile"):
            return False
        lib.axon_start_nrt_profile.argtypes = [
            ctypes.POINTER(ctypes.c_int64),
            ctypes.c_size_t,
        ]
        lib.axon_start_nrt_profile.restype = ctypes.c_int64
        lib.axon_stop_nrt_profile.argtypes = [ctypes.c_char_p]
        lib.axon_stop_nrt_profile.restype = ctypes.c_int64

        @contextlib.contextmanager
        def hook(output_dir, device_ids):
            import jax

            jax.devices()
            if device_ids:
                ids = (ctypes.c_int64 * len(device_ids))(*device_ids)
                rc = lib.axon_start_nrt_profile(ids, len(device_ids))
            else:
                rc = lib.axon_start_nrt_profile(None, 0)
            if rc != 0:
                raise RuntimeError(f"axon_start_nrt_profile rc={rc}")
            try:
                yield
            finally:
                lib.axon_stop_nrt_profile(str(output_dir).encode())

        mod = types.ModuleType("antenv.axon_hooks")
        mod.get_axon_ntff_profile_hook = lambda: hook
        mod.set_axon_ntff_profile_hook = lambda h: None
        sys.modules["antenv.axon_hooks"] = mod
        bu.upload_artifacts = lambda tmpdir: f"file://{tmpdir}"
        return True
    except Exception:
        return False


def kernel(x, gate_w, w1, w2, w3):
    from concourse.bass_utils import run_bass_kernel_spmd

    global N_TOK, CAP
    B, S, D = x.shape
    xf = np.asarray(x, np.float32).reshape(-1, D)
    N_TOK = xf.shape[0]
    CAP = int(N_TOK / E * 1.25)

    top_idx, top_w, aux_loss = _router_host(xf, gate_w)

    # Integer bookkeeping (exact): rank of each token in its expert queue.
    idxs, poss, keeps = [], [], []
    kept_cnt = np.zeros((TOP_K, E), np.int64)
    for k in range(TOP_K):
        idx = top_idx[:, k]
        oh = (idx[:, None] == np.arange(E)[None, :]).astype(np.int32)
        pos = oh.cumsum(0)[np.arange(N_TOK), idx] - 1
        keep = pos < CAP
        kept_cnt[k] = np.bincount(idx[keep], minlength=E)
        idxs.append(idx)
        poss.append(pos)
        keeps.append(keep)

    # Tight per-expert packing: rows [k=0 kept | k=1 kept]; program sized
    # to the exact max rows across experts (ragged tail handled on device).
    rows_e = kept_cnt.sum(0)
    pr = max(int(rows_e.max()), 128)

    packed = np.zeros((E, pr, D), np.float32)
    for k in range(TOP_K):
        idx, pos, keep = idxs[k], poss[k], keeps[k]
        row = pos + (kept_cnt[0][idx] if k == 1 else 0)
        packed[idx[keep], row[keep]] = xf[keep]

    # Per-core device inputs (expert-parallel).
    in_maps = []
    for e in range(E):
        xt = np.ascontiguousarray(packed[e].T).astype(BF16)  # [D, pr]
        w1sh = (
            np.asarray(w1[e], np.float32)
            .reshape(FC, 128, DC, 128)
            .transpose(0, 3, 2, 1)
            .reshape(FC, 128, D_MODEL)
        )
        w3sh = (
            np.asarray(w3[e], np.float32)
            .reshape(FC, 128, DC, 128)
            .transpose(0, 3, 2, 1)
            .reshape(FC, 128, D_MODEL)
        )
        w13 = np.ascontiguousarray(np.stack([w1sh, w3sh])).astype(BF16)
        w2t = np.ascontiguousarray(np.asarray(w2[e], np.float32).T).astype(BF16)
        in_maps.append({"xt": xt, "w13": w13, "w2t": w2t})

    if pr not in _NC_CACHE:
        _NC_CACHE[pr] = _build_program(pr)
    nc = _NC_CACHE[pr]

    trace = os.environ.get("BASS_KERNEL_TRACE") == "1"
    kwargs = {}
    if trace and _install_trace_shim():
        kwargs = {"trace": True, "tmpdir": os.environ.get("BASS_KERNEL_TRACE_DIR")}
    res = run_bass_kernel_spmd(nc, in_maps, list(range(E)), **kwargs)
    if trace:
        print(f"HW exec time: {res.exec_time_ns} ns")

    obs = np.stack([res.results[e]["ob"] for e in range(E)])  # [E, pr, D] f32

    out = np.zeros_like(xf)
    for k in range(TOP_K):
        idx, pos, keep = idxs[k], poss[k], keeps[k]
        row = pos + (kept_cnt[0][idx] if k == 1 else 0)
        row = np.where(keep, row, 0)
        gathered = obs[idx, row]  # [N, D]
        coef = (keep.astype(np.float32) * top_w[:, k])[:, None]
        out += gathered * coef

    output = out.reshape(B, S, D)
    return output, np.float32(aux_loss)


# revision 19
# speedup vs baseline: 1.0498x; 1.0069x over previous
"""MoE layer (8 experts, top-2, capacity 1280) on 8 Trainium2 NeuronCores.

Sharding: expert-parallel. The router (softmax/top-k/position bookkeeping,
~0.3% of FLOPs) runs on host exactly mirroring the reference ops; the
dispatched rows are packed tightly per expert on host (we hold the full
input anyway, so no all-to-all is needed) and core e runs expert e's SwiGLU
FFN over its [padded_rows, d_model] buffer — perfectly load balanced, and
only real routed rows (rounded up to 128) are computed instead of the full
2*capacity zero-padded buffer. Matmuls run in bf16 with fp32 PSUM
accumulation.
"""

import os

import numpy as np
import ml_dtypes

D_MODEL = 1024
D_FF = 4096
E = 8
TOP_K = 2
CAP = 1280  # int(8192 / 8 * 1.25)
N_TOK = 8192
FC = D_FF // 128  # 32 f-chunks
DC = D_MODEL // 128  # 8 d-chunks

BF16 = ml_dtypes.bfloat16

_NC_CACHE = {}  # padded_rows -> compiled Bass program


def _blocks_for(pr):
    # Blocks should be >= 256 wide: a narrow block restreams the full
    # 16.8MB of w1/w3 for little PE work and becomes weight-DMA-bound
    # (PE starves, HAM re-throttles). Widths need not be multiples of
    # 128 — stage B handles a ragged tail sub-block.
    n512, rem = divmod(pr, 512)
    if rem == 0:
        return [512] * n512
    if rem >= 256 or n512 == 0:
        return [512] * n512 + [rem]
    return [512] * (n512 - 1) + [256, 256 + rem]


def _build_program(pr):
    import concourse.bacc as bacc
    import concourse.mybir as mybir
    import concourse.tile as tile

    f32 = mybir.dt.float32
    bf16 = mybir.dt.bfloat16
    blocks = _blocks_for(pr)

    nc = bacc.Bacc("TRN2", target_bir_lowering=False, debug=False, num_devices=E)
    xt_d = nc.dram_tensor("xt", [D_MODEL, pr], bf16, kind="ExternalInput")
    w13_d = nc.dram_tensor("w13", [2, FC, 128, D_MODEL], bf16, kind="ExternalInput")
    w2t_d = nc.dram_tensor("w2t", [D_FF, D_MODEL], bf16, kind="ExternalInput")
    ob_d = nc.dram_tensor("ob", [pr, D_MODEL], f32, kind="ExternalOutput")

    with tile.TileContext(nc) as tc:
        with (
            tc.tile_pool(name="pxt", bufs=1) as pxt,
            tc.tile_pool(name="pw2", bufs=1) as pw2,
            tc.tile_pool(name="ph", bufs=1) as ph,
            tc.tile_pool(name="pw", bufs=6) as pw,
            tc.tile_pool(name="ps", bufs=2) as ps,
            tc.tile_pool(name="po", bufs=3) as po,
            tc.tile_pool(name="pps", bufs=2, space="PSUM") as pps,
            tc.tile_pool(name="ppo", bufs=2, space="PSUM") as ppo,
        ):
            # Resident: dispatched tokens transposed, [p, dc, c].
            # Loaded in per-token-block strips so the first matmul isn't
            # gated on the full transfer.
            xtsb = pxt.tile([128, DC, pr], bf16)
            xt_src = xt_d.ap().rearrange("(a p) c -> p a c", p=128)

            def load_xt_strip(c0, w, split=False):
                if split:
                    # per-dc transfers: lets the first accumulation group's
                    # matmul(dc) start as soon as slice dc lands
                    for dc in range(DC):
                        nc.sync.dma_start(
                            xtsb[:, dc, c0 : c0 + w],
                            xt_src[:, dc, c0 : c0 + w],
                        )
                else:
                    nc.sync.dma_start(
                        xtsb[:, :, c0 : c0 + w], xt_src[:, :, c0 : c0 + w]
                    )

            # Resident: w2^T, [p, fc, d] — trickled in during block 0 stage A
            # (first needed at block 0 stage B).
            w2sb = pw2.tile([128, FC, D_MODEL], bf16)
            w2_src = w2t_d.ap().rearrange("(a p) d -> p a d", p=128)

            load_xt_strip(0, blocks[0])
            c0 = 0
            for bi, W in enumerate(blocks):
                h = ph.tile([128, FC, 512], bf16)
                for fc in range(FC):
                    if fc == 2 and bi + 1 < len(blocks):
                        # prefetch next strip; emitted after fc0/fc1 weight
                        # loads so block 0's critical bytes win queue order
                        load_xt_strip(c0 + W, blocks[bi + 1])
                    if bi == 0 and 8 <= fc < 16:
                        s = fc - 8
                        nc.sync.dma_start(
                            w2sb[:, s * 4 : (s + 1) * 4, :],
                            w2_src[:, s * 4 : (s + 1) * 4, :],
                        )
                    w1t = pw.tile([128, D_MODEL], bf16)
                    nc.sync.dma_start(w1t[:], w13_d.ap()[0, fc])
                    w3t = pw.tile([128, D_MODEL], bf16)
                    nc.sync.dma_start(w3t[:], w13_d.ap()[1, fc])
                    p1 = pps.tile([128, 512], f32)
                    p3 = pps.tile([128, 512], f32)
                    for dc in range(DC):
                        nc.tensor.matmul(
                            p1[:, :W],
                            w1t[:, dc * 128 : (dc + 1) * 128],
                            xtsb[:, dc, c0 : c0 + W],
                            start=(dc == 0),
                            stop=(dc == DC - 1),
                        )
                    for dc in range(DC):
                        nc.tensor.matmul(
                            p3[:, :W],
                            w3t[:, dc * 128 : (dc + 1) * 128],
                            xtsb[:, dc, c0 : c0 + W],
                            start=(dc == 0),
                            stop=(dc == DC - 1),
                        )
                    s = ps.tile([128, 512], f32)
                    nc.scalar.activation(
                        s[:, :W], p1[:, :W], mybir.ActivationFunctionType.Silu
                    )
                    nc.vector.tensor_mul(h[:, fc, :W], s[:, :W], p3[:, :W])
                # Stage B: ob[t, d] = sum_f h[f, t] * w2t[f, d]
                # (ragged tail sub-block allowed: matmul cost depends on the
                # moving free dim, not the output partition count)
                for ts4 in range(-(-W // 128)):
                    t0 = ts4 * 128
                    tw = min(128, W - t0)
                    for dh in range(2):
                        pob = ppo.tile([128, 512], f32)
                        for fc in range(FC):
                            nc.tensor.matmul(
                                pob[:tw, :],
                                h[:, fc, t0 : t0 + tw],
                                w2sb[:, fc, dh * 512 : (dh + 1) * 512],
                                start=(fc == 0),
                                stop=(fc == FC - 1),
                            )
                        ot = po.tile([128, 512], f32)
                        nc.vector.tensor_copy(ot[:tw, :], pob[:tw, :])
                        r0 = c0 + t0
                        nc.sync.dma_start(
                            ob_d.ap()[r0 : r0 + tw, dh * 512 : (dh + 1) * 512],
                            ot[:tw, :],
                        )
                c0 += W

    nc.compile()
    return nc


def _router_host(xf, gate_w):
    """Router math, mirroring the reference ops on jax-CPU for exactness."""
    try:
        import jax
        import jax.numpy as jnp

        cpu = jax.devices("cpu")[0]
        with jax.default_device(cpu):
            router_logits = jnp.asarray(xf) @ jnp.asarray(gate_w).T
            router_probs = jax.nn.softmax(router_logits, axis=-1)
            top_probs, top_idx = jax.lax.top_k(router_probs, TOP_K)
            top_w = top_probs / (top_probs.sum(-1, keepdims=True) + 1e-10)

            erange = jnp.arange(E, dtype=top_idx.dtype)
            counts = jnp.sum(
                (top_idx[..., None] == erange[None, None, :]).astype(jnp.float32),
                axis=(0, 1),
            )
            f = counts / (N_TOK * TOP_K)
            P = router_probs.mean(axis=0)
            aux_loss = E * jnp.sum(f * P)
        return (
            np.asarray(top_idx),
            np.asarray(top_w),
            np.asarray(aux_loss),
        )
    except Exception:
        # numpy fallback (same math; top-k ties broken by lowest index)
        logits = xf @ np.asarray(gate_w, np.float32).T
        z = logits - logits.max(-1, keepdims=True)
        ez = np.exp(z)
        probs = ez / ez.sum(-1, keepdims=True)
        order = np.argsort(-probs, axis=-1, kind="stable")
        top_idx = order[:, :TOP_K].astype(np.int32)
        top_probs = np.take_along_axis(probs, top_idx, axis=-1)
        top_w = top_probs / (top_probs.sum(-1, keepdims=True) + 1e-10)
        counts = np.bincount(top_idx.ravel(), minlength=E).astype(np.float32)
        f = counts / (N_TOK * TOP_K)
        P = probs.mean(axis=0)
        aux_loss = np.float32(E * np.sum(f * P))
        return top_idx, top_w.astype(np.float32), aux_loss


def _install_trace_shim():
    """Dev-only: register the NTFF profile hook (missing antenv.axon_hooks)
    so run_bass_kernel_spmd(trace=True) can capture HW exec time under axon.
    Returns True if tracing is usable."""
    try:
        import contextlib
        import ctypes
        import sys
        import types

        import concourse.bass_utils as bu

        try:
            from antenv.axon_hooks import get_axon_ntff_profile_hook  # noqa: F401

            return True  # real hooks present
        except ImportError:
            pass

        lib = ctypes.CDLL("/opt/axon/libaxon_pjrt.so")
        if not hasattr(lib, "axon_start_nrt_profile"):
            return False
        lib.axon_start_nrt_profile.argtypes = [
            ctypes.POINTER(ctypes.c_int64),
            ctypes.c_size_t,
        ]
        lib.axon_start_nrt_profile.restype = ctypes.c_int64
        lib.axon_stop_nrt_profile.argtypes = [ctypes.c_char_p]
        lib.axon_stop_nrt_profile.restype = ctypes.c_int64

        @contextlib.contextmanager
        def hook(output_dir, device_ids):
            import jax

            jax.devices()
            if device_ids:
                ids = (ctypes.c_int64 * len(device_ids))(*device_ids)
                rc = lib.axon_start_nrt_profile(ids, len(device_ids))
            else:
                rc = lib.axon_start_nrt_profile(None, 0)
            if rc != 0:
                raise RuntimeError(f"axon_start_nrt_profile rc={rc}")
            try:
                yield
            finally:
                lib.axon_stop_nrt_profile(str(output_dir).encode())

        mod = types.ModuleType("antenv.axon_hooks")
        mod.get_axon_ntff_profile_hook = lambda: hook
        mod.set_axon_ntff_profile_hook = lambda h: None
        sys.modules["antenv.axon_hooks"] = mod
        bu.upload_artifacts = lambda tmpdir: f"file://{tmpdir}"
        return True
    except Exception:
        return False


def kernel(x, gate_w, w1, w2, w3):
    from concourse.bass_utils import run_bass_kernel_spmd

    global N_TOK, CAP
    B, S, D = x.shape
    xf = np.asarray(x, np.float32).reshape(-1, D)
    N_TOK = xf.shape[0]
    CAP = int(N_TOK / E * 1.25)

    top_idx, top_w, aux_loss = _router_host(xf, gate_w)

    # Integer bookkeeping (exact): rank of each token in its expert queue.
    idxs, poss, keeps = [], [], []
    kept_cnt = np.zeros((TOP_K, E), np.int64)
    for k in range(TOP_K):
        idx = top_idx[:, k]
        oh = (idx[:, None] == np.arange(E)[None, :]).astype(np.int32)
        pos = oh.cumsum(0)[np.arange(N_TOK), idx] - 1
        keep = pos < CAP
        kept_cnt[k] = np.bincount(idx[keep], minlength=E)
        idxs.append(idx)
        poss.append(pos)
        keeps.append(keep)

    # Tight per-expert packing: rows [k=0 kept | k=1 kept]; program sized
    # to the exact max rows across experts (ragged tail handled on device).
    rows_e = kept_cnt.sum(0)
    pr = max(int(rows_e.max()), 128)

    packed = np.zeros((E, pr, D), np.float32)
    for k in range(TOP_K):
        idx, pos, keep = idxs[k], poss[k], keeps[k]
        row = pos + (kept_cnt[0][idx] if k == 1 else 0)
        packed[idx[keep], row[keep]] = xf[keep]

    # Per-core device inputs (expert-parallel).
    in_maps = []
    for e in range(E):
        xt = np.ascontiguousarray(packed[e].T).astype(BF16)  # [D, pr]
        w1sh = (
            np.asarray(w1[e], np.float32)
            .reshape(FC, 128, DC, 128)
            .transpose(0, 3, 2, 1)
            .reshape(FC, 128, D_MODEL)
        )
        w3sh = (
            np.asarray(w3[e], np.float32)
            .reshape(FC, 128, DC, 128)
            .transpose(0, 3, 2, 1)
            .reshape(FC, 128, D_MODEL)
        )
        w13 = np.ascontiguousarray(np.stack([w1sh, w3sh])).astype(BF16)
        w2t = np.ascontiguousarray(np.asarray(w2[e], np.float32).T).astype(BF16)
        in_maps.append({"xt": xt, "w13": w13, "w2t": w2t})

    if pr not in _NC_CACHE:
        _NC_CACHE[pr] = _build_program(pr)
    nc = _NC_CACHE[pr]

    trace = os.environ.get("BASS_KERNEL_TRACE") == "1"
    kwargs = {}
    if trace and _install_trace_shim():
        kwargs = {"trace": True, "tmpdir": os.environ.get("BASS_KERNEL_TRACE_DIR")}
    res = run_bass_kernel_spmd(nc, in_maps, list(range(E)), **kwargs)
    if trace:
        print(f"HW exec time: {res.exec_time_ns} ns")

    obs = np.stack([res.results[e]["ob"] for e in range(E)])  # [E, pr, D] f32

    out = np.zeros_like(xf)
    for k in range(TOP_K):
        idx, pos, keep = idxs[k], poss[k], keeps[k]
        row = pos + (kept_cnt[0][idx] if k == 1 else 0)
        row = np.where(keep, row, 0)
        gathered = obs[idx, row]  # [N, D]
        coef = (keep.astype(np.float32) * top_w[:, k])[:, None]
        out += gathered * coef

    output = out.reshape(B, S, D)
    return output, np.float32(aux_loss)
